# revision 10
# baseline (speedup 1.0000x reference)
"""Grouped-expert FFN (MoE) Trainium2 kernel.

Problem: E=64 experts, each x[1024,512] @ w1[512,2048] -> +b1 -> gelu(erf)
-> @ w2[2048,512] -> +b2, rows >= valid_load[e] zeroed.

Strategy (v2 — unit-based, bf16):
 - Work is decomposed into column "units": every core runs the same static
   sequence of unit widths (SPMD), but the host assigns ANY (expert,
   column-range) piece to each (core, unit) cell, with a per-unit copy of
   that expert's weights in DRAM. This removes the per-slot max-over-cores
   padding of expert-parallel layouts: ~4800 columns/core vs 5472.
 - The unit width multiset is optimized at runtime by a deterministic
   annealer over per-expert cuts (rank-deal dominance: pieces sorted desc,
   unit j = max of piece ranks [8j, 8j+8)).
 - All matmul operands are bf16 (PE streams bf16 at 1 elem/cell/cycle,
   identical peak to fp32r, but half the HBM traffic; PSUM accumulates
   fp32). rel err ~3e-3 vs the 2e-2 gate.
 - Host transposes x per expert (xT [D,C]) so the device contracts over D
   with no on-chip transposes; both biases land on the partition axis ->
   free via ACT activation bias. GEMM1: hT = w1-tiles.T @ xT, GEMM2:
   yT = w2-tiles.T @ hT.
 - Unit 0's w1 is DMA'd in 16 m-slices so the first matmul starts ~2us
   after queue init instead of waiting for the full 2MB tile; y is written
   back per m-tile to shrink the kernel tail.
"""

import random

import numpy as np

import concourse.bass as bass
import concourse.bacc as bacc
import concourse.tile as tile
from concourse import mybir
from concourse.bass_utils import run_bass_kernel_spmd

E, CAP, D, H = 64, 1024, 512, 2048
N_CORES = 8
KT1, MT1 = D // 128, H // 128     # GEMM1: 4 contraction tiles, 16 out tiles
KT2, MT2 = H // 128, D // 128     # GEMM2: 16 contraction tiles, 4 out tiles
WMAX = 512                        # PSUM bank = 512 fp32 columns

F32 = mybir.dt.float32
BF16 = mybir.dt.bfloat16

_PROGRAM_CACHE: dict[tuple, object] = {}
_SCHEDULE_CACHE: dict[tuple, tuple] = {}
LAST_RESULT = None               # test harness introspection


# ----------------------------- scheduling -----------------------------

def _unit_pe_ns(w):
    # warm PE: N cycles @2.4GHz + ~22ns MM+LDWEIGHTS dispatch overhead
    # (HW-measured: W=480 paces at 227ns vs 200ns stream)
    return 128 * (max(w, 64) / 2.4 + 22.0)


def _unit_dma_ns(w):
    # per-unit HBM bytes: w1+w2 bf16 + x bf16 + y f32, at ~400 B/ns
    return (2 * (D * H * 2) + D * w * 2 + D * w * 4) / 400.0


def _units_of(cuts):
    pieces = []
    for ws in cuts:
        pieces += ws
    pieces.sort(reverse=True)
    return [pieces[j] for j in range(0, len(pieces), 8)]


def _sched_cost(cuts):
    u = _units_of(cuts)
    return max(sum(_unit_pe_ns(w) for w in u), sum(_unit_dma_ns(w) for w in u))


def _initial_cuts(v):
    cuts = []
    for ve in v:
        ve = int(ve)
        if ve <= 0:
            cuts.append([])
            continue
        k = -(-ve // WMAX)
        w = max(16, -(-(-(-ve // k)) // 16) * 16)
        ws = [w] * k
        over = sum(ws) - ve
        t = over // 16 * 16
        i = len(ws) - 1
        while t > 0 and i >= 0:
            d = min(t, ws[i] - 16)
            ws[i] -= d
            t -= d
            i -= 1
        ws.sort(reverse=True)
        cuts.append(ws)
    return cuts


def _anneal(v, iters=150000):
    rng = random.Random(3)
    cur = _initial_cuts(v)
    cur_c = _sched_cost(cur)
    best = [ws[:] for ws in cur]
    best_c = cur_c
    temp = 2000.0
    for _ in range(iters):
        temp = max(10.0, temp * 0.99997)
        e = rng.randrange(len(v))
        if v[e] <= 0:
            continue
        ws = cur[e][:]
        op = rng.random()
        if op < 0.45 and ws:
            i = rng.randrange(len(ws))
            if rng.random() < 0.5 and sum(ws) - 16 >= v[e] and ws[i] > 16:
                ws[i] -= 16
            else:
                if len(ws) < 2:
                    continue
                j = rng.randrange(len(ws))
                if i == j or ws[i] <= 16 or ws[j] >= WMAX:
                    continue
                ws[i] -= 16
                ws[j] += 16
        elif op < 0.65 and ws:
            i = rng.randrange(len(ws))
            if ws[i] < 32:
                continue
            a = rng.randrange(1, ws[i] // 16) * 16
            ws = ws[:i] + [a, ws[i] - a] + ws[i + 1:]
        elif len(ws) >= 2:
            i = rng.randrange(len(ws))
            j = rng.randrange(len(ws))
            if i == j or ws[i] + ws[j] > WMAX:
                continue
            ws = [w for k2, w in enumerate(ws) if k2 not in (i, j)] + [ws[i] + ws[j]]
        else:
            continue
        if not all(16 <= w <= WMAX for w in ws) or sum(ws) < v[e]:
            continue
        ws.sort(reverse=True)
        old = cur[e]
        cur[e] = ws
        new_c = _sched_cost(cur)
        if new_c <= cur_c or rng.random() < np.exp((cur_c - new_c) / temp):
            cur_c = new_c
            if new_c < best_c:
                best_c = new_c
                best = [w[:] for w in cur]
        else:
            cur[e] = old
    return best


def _schedule(v_key):
    """v (tuple of 64 ints) -> (unit widths desc, assign[j][c] = (e, c0, cov)
    or None). All cores share the width sequence; piece rank r (pieces sorted
    desc) maps to unit r//8, core r%8, so piece width <= unit width."""
    if v_key in _SCHEDULE_CACHE:
        return _SCHEDULE_CACHE[v_key]
    v = list(v_key)
    cuts = _anneal(v)
    pieces = []  # (width, expert)
    for e, ws in enumerate(cuts):
        for w in ws:
            pieces.append((w, e))
    pieces.sort(key=lambda p: (-p[0], p[1]))
    widths = tuple(pieces[j][0] for j in range(0, len(pieces), 8))
    nunits = len(widths)
    # slots per expert: (unit_width, unit_idx, core)
    slots = {e: [] for e in range(len(v))}
    for r, (w, e) in enumerate(pieces):
        slots[e].append((widths[r // 8], r // 8, r % 8))
    assign = [[None] * N_CORES for _ in range(nunits)]
    for e, sl in slots.items():
        sl.sort(key=lambda s: -s[0])  # widest first
        cum = 0
        for wu, j, c in sl:
            if cum >= v[e]:
                continue  # overshoot slot -> dummy
            c0 = min(cum, CAP - wu)
            end = min(c0 + wu, v[e])
            assign[j][c] = (e, c0, end - c0)
            cum = end
        assert cum >= v[e], (e, v[e], sl)
    _SCHEDULE_CACHE[v_key] = (widths, assign)
    return widths, assign


# ----------------------------- device program -----------------------------

def _build_program(widths: tuple):
    nc = bacc.Bacc(None, target_bir_lowering=False)
    K = len(widths)

    xt = nc.dram_tensor("xt", [K, D, WMAX], BF16, kind="ExternalInput")
    w1g = nc.dram_tensor("w1g", [K, D, H], BF16, kind="ExternalInput")
    w2g = nc.dram_tensor("w2g", [K, H, D], BF16, kind="ExternalInput")
    b1g = nc.dram_tensor("b1g", [K, 128, MT1], F32, kind="ExternalInput")
    b2g = nc.dram_tensor("b2g", [K, 128, MT2], F32, kind="ExternalInput")
    yt = nc.dram_tensor("yt", [K, D, WMAX], F32, kind="ExternalOutput")
    scr = nc.dram_tensor("scr", [128, 64], F32, kind="ExternalOutput")

    Gelu = mybir.ActivationFunctionType.Gelu
    Ident = mybir.ActivationFunctionType.Identity

    # emit units ascending by width: the smallest unit's x+w arrive fastest
    # (compute starts early) and each unit's growing compute window amortizes
    # the constant 4.2MB/unit weight-DMA debt without starving the PE
    emit_order = list(range(K - 1, -1, -1))

    with tile.TileContext(nc) as tc:
        with (
            tc.tile_pool(name="wu", bufs=1) as wu,
            tc.tile_pool(name="w1p", bufs=2) as w1p,
            tc.tile_pool(name="w2p", bufs=2) as w2p,
            tc.tile_pool(name="bp", bufs=2) as bp,
            tc.tile_pool(name="xp", bufs=3) as xp,
            tc.tile_pool(name="hp", bufs=2) as hp,
            tc.tile_pool(name="yp", bufs=2) as yp,
            tc.tile_pool(name="ps_h", bufs=4, space="PSUM") as ps_h,
            tc.tile_pool(name="ps_y", bufs=4, space="PSUM") as ps_y,
        ):
            # PE pre-warm: ~5us of dummy matmuls during the initial DMA wait
            # flips the HAM clock gate to 8/8 before real work arrives
            warm = wu.tile([128, 64], BF16, tag="warm")
            nc.gpsimd.memset(warm, 0.0)
            wps = None
            for _ in range(96):
                wps = ps_h.tile([128, 64], F32, tag="psh")
                nc.tensor.matmul(
                    wps[:64, :], lhsT=warm, rhs=warm, start=True, stop=True)
            wout = wu.tile([128, 64], F32, tag="wout")
            nc.scalar.activation(wout[:64, :], wps[:64, :], Ident)
            nc.gpsimd.dma_start(out=scr[:64, :], in_=wout[:64, :])

            for j in emit_order:
                W = widths[j]
                first = j == K - 1
                w1_src = w1g[j].rearrange("(k p) h -> p k h", p=128)
                w1_t = w1p.tile([128, KT1, H], BF16, tag="w1")
                x_t = xp.tile([128, KT1, WMAX], BF16, tag="x")
                if first:
                    # fine-grained first load: compute starts after m-slice 0
                    nc.sync.dma_start(
                        out=w1_t[:, :, 0:128], in_=w1_src[:, :, 0:128])
                    nc.sync.dma_start(
                        out=x_t[:, :, :W],
                        in_=xt[j].rearrange("(k p) c -> p k c", p=128)[:, :, :W])
                    for m in range(1, MT1):
                        nc.sync.dma_start(
                            out=w1_t[:, :, m * 128:(m + 1) * 128],
                            in_=w1_src[:, :, m * 128:(m + 1) * 128])
                else:
                    nc.sync.dma_start(out=w1_t, in_=w1_src)
                    nc.sync.dma_start(
                        out=x_t[:, :, :W],
                        in_=xt[j].rearrange("(k p) c -> p k c", p=128)[:, :, :W])
                b1_t = bp.tile([128, MT1], F32, tag="b1")
                nc.scalar.dma_start(out=b1_t, in_=b1g[j])
                b2_t = bp.tile([128, MT2], F32, tag="b2")
                nc.scalar.dma_start(out=b2_t, in_=b2g[j])
                # w2 rides the second HWDGE ring (ACT): needed only for GEMM2
                w2_t = w2p.tile([128, KT2, D], BF16, tag="w2")
                nc.scalar.dma_start(
                    out=w2_t, in_=w2g[j].rearrange("(k p) d -> p k d", p=128))

                h_t = hp.tile([128, KT2, WMAX], BF16, tag="h")
                for m in range(MT1):
                    ps = ps_h.tile([128, WMAX], F32, tag="psh")
                    for k in range(KT1):
                        nc.tensor.matmul(
                            ps[:, :W],
                            lhsT=w1_t[:, k, m * 128:(m + 1) * 128],
                            rhs=x_t[:, k, :W],
                            start=(k == 0),
                            stop=(k == KT1 - 1),
                        )
                    nc.scalar.activation(
                        h_t[:, m, :W], ps[:, :W], Gelu, bias=b1_t[:, m:m + 1])

                y_t = yp.tile([128, MT2, WMAX], F32, tag="y")
                yt_s = yt[j].rearrange("(m p) c -> p m c", p=128)
                last = j == 0
                for dm in range(MT2):
                    ps2 = ps_y.tile([128, WMAX], F32, tag="psy")
                    for k in range(KT2):
                        nc.tensor.matmul(
                            ps2[:, :W],
                            lhsT=w2_t[:, k, dm * 128:(dm + 1) * 128],
                            rhs=h_t[:, k, :W],
                            start=(k == 0),
                            stop=(k == KT2 - 1),
                        )
                    nc.scalar.activation(
                        y_t[:, dm, :W], ps2[:, :W], Ident, bias=b2_t[:, dm:dm + 1])
                    if last:
                        # final unit: write out per m-tile to shrink the tail
                        nc.gpsimd.dma_start(
                            out=yt_s[:, dm, :W], in_=y_t[:, dm, :W])
                if not last:
                    nc.gpsimd.dma_start(
                        out=yt_s[:, :, :W], in_=y_t[:, :, :W])

    nc.compile()
    return nc


# ----------------------------- host wrapper -----------------------------

def kernel(packed_inputs, valid_load, w1, b1, w2, b2, _trace=False, **_):
    global LAST_RESULT
    packed_inputs = np.asarray(packed_inputs, np.float32)
    w1 = np.asarray(w1, np.float32)
    b1 = np.asarray(b1, np.float32)
    w2 = np.asarray(w2, np.float32)
    b2 = np.asarray(b2, np.float32)
    v = np.asarray(valid_load).astype(np.int64)

    out = np.zeros((E, CAP, D), np.float32)
    if int(v.max()) <= 0:
        return out

    widths, assign = _schedule(tuple(int(x) for x in v))
    K = len(widths)

    if widths not in _PROGRAM_CACHE:
        _PROGRAM_CACHE[widths] = _build_program(widths)
    nc = _PROGRAM_CACHE[widths]

    bf16 = mybir.dt.np(BF16)
    xt_all = np.ascontiguousarray(
        packed_inputs.transpose(0, 2, 1)).astype(bf16)      # [E, D, CAP]
    w1b = w1.astype(bf16)
    w2b = w2.astype(bf16)
    b1r = np.ascontiguousarray(
        b1.reshape(E, MT1, 128).transpose(0, 2, 1))          # [E, 128, MT1]
    b2r = np.ascontiguousarray(
        b2.reshape(E, MT2, 128).transpose(0, 2, 1))

    in_maps = []
    for c in range(N_CORES):
        xtc = np.zeros((K, D, WMAX), bf16)
        w1c = np.zeros((K, D, H), bf16)
        w2c = np.zeros((K, H, D), bf16)
        b1c = np.zeros((K, 128, MT1), np.float32)
        b2c = np.zeros((K, 128, MT2), np.float32)
        for j, W in enumerate(widths):
            pc = assign[j][c]
            if pc is None:
                continue
            e, c0, _cov = pc
            xtc[j, :, :W] = xt_all[e][:, c0:c0 + W]
            w1c[j] = w1b[e]
            w2c[j] = w2b[e]
            b1c[j] = b1r[e]
            b2c[j] = b2r[e]
        in_maps.append({"xt": xtc, "w1g": w1c, "w2g": w2c,
                        "b1g": b1c, "b2g": b2c})

    res = run_bass_kernel_spmd(nc, in_maps, list(range(N_CORES)), trace=_trace)
    LAST_RESULT = res

    for c in range(N_CORES):
        ytc = res.results[c]["yt"]
        for j in range(K):
            pc = assign[j][c]
            if pc is None:
                continue
            e, c0, cov = pc
            out[e, c0:c0 + cov, :] = ytc[j][:, :cov].T
    return out


# revision 12
# speedup vs baseline: 1.0161x; 1.0161x over previous
"""Grouped-expert FFN (MoE) Trainium2 kernel.

Problem: E=64 experts, each x[1024,512] @ w1[512,2048] -> +b1 -> gelu(erf)
-> @ w2[2048,512] -> +b2, rows >= valid_load[e] zeroed.

Strategy (v2 — unit-based, bf16):
 - Work is decomposed into column "units": every core runs the same static
   sequence of unit widths (SPMD), but the host assigns ANY (expert,
   column-range) piece to each (core, unit) cell, with a per-unit copy of
   that expert's weights in DRAM. This removes the per-slot max-over-cores
   padding of expert-parallel layouts: ~4800 columns/core vs 5472.
 - The unit width multiset is optimized at runtime by a deterministic
   annealer over per-expert cuts (rank-deal dominance: pieces sorted desc,
   unit j = max of piece ranks [8j, 8j+8)).
 - All matmul operands are bf16 (PE streams bf16 at 1 elem/cell/cycle,
   identical peak to fp32r, but half the HBM traffic; PSUM accumulates
   fp32). rel err ~3e-3 vs the 2e-2 gate.
 - Host transposes x per expert (xT [D,C]) so the device contracts over D
   with no on-chip transposes; both biases land on the partition axis ->
   free via ACT activation bias. GEMM1: hT = w1-tiles.T @ xT, GEMM2:
   yT = w2-tiles.T @ hT.
 - Unit 0's w1 is DMA'd in 16 m-slices so the first matmul starts ~2us
   after queue init instead of waiting for the full 2MB tile; y is written
   back per m-tile to shrink the kernel tail.
"""

import random

import numpy as np

import concourse.bass as bass
import concourse.bacc as bacc
import concourse.tile as tile
from concourse import mybir
from concourse.bass_utils import run_bass_kernel_spmd

E, CAP, D, H = 64, 1024, 512, 2048
N_CORES = 8
KT1, MT1 = D // 128, H // 128     # GEMM1: 4 contraction tiles, 16 out tiles
KT2, MT2 = H // 128, D // 128     # GEMM2: 16 contraction tiles, 4 out tiles
WMAX = 512                        # PSUM bank = 512 fp32 columns

F32 = mybir.dt.float32
BF16 = mybir.dt.bfloat16

_PROGRAM_CACHE: dict[tuple, object] = {}
_SCHEDULE_CACHE: dict[tuple, tuple] = {}
LAST_RESULT = None               # test harness introspection


# ----------------------------- scheduling -----------------------------

def _unit_pe_ns(w):
    # warm PE: N cycles @2.4GHz + ~22ns MM+LDWEIGHTS dispatch overhead
    # (HW-measured: W=480 paces at 227ns vs 200ns stream)
    return 128 * (max(w, 64) / 2.4 + 22.0)


def _unit_dma_ns(w):
    # per-unit HBM bytes: w1+w2 bf16 + x bf16 + y f32, at ~400 B/ns
    return (2 * (D * H * 2) + D * w * 2 + D * w * 4) / 400.0


def _units_of(cuts):
    pieces = []
    for ws in cuts:
        pieces += ws
    pieces.sort(reverse=True)
    return [pieces[j] for j in range(0, len(pieces), 8)]


def _sched_cost(cuts):
    u = _units_of(cuts)
    return max(sum(_unit_pe_ns(w) for w in u), sum(_unit_dma_ns(w) for w in u))


def _initial_cuts(v):
    cuts = []
    for ve in v:
        ve = int(ve)
        if ve <= 0:
            cuts.append([])
            continue
        k = -(-ve // WMAX)
        w = max(16, -(-(-(-ve // k)) // 16) * 16)
        ws = [w] * k
        over = sum(ws) - ve
        t = over // 16 * 16
        i = len(ws) - 1
        while t > 0 and i >= 0:
            d = min(t, ws[i] - 16)
            ws[i] -= d
            t -= d
            i -= 1
        ws.sort(reverse=True)
        cuts.append(ws)
    return cuts


def _anneal(v, iters=150000):
    rng = random.Random(3)
    cur = _initial_cuts(v)
    cur_c = _sched_cost(cur)
    best = [ws[:] for ws in cur]
    best_c = cur_c
    temp = 2000.0
    for _ in range(iters):
        temp = max(10.0, temp * 0.99997)
        e = rng.randrange(len(v))
        if v[e] <= 0:
            continue
        ws = cur[e][:]
        op = rng.random()
        if op < 0.45 and ws:
            i = rng.randrange(len(ws))
            if rng.random() < 0.5 and sum(ws) - 16 >= v[e] and ws[i] > 16:
                ws[i] -= 16
            else:
                if len(ws) < 2:
                    continue
                j = rng.randrange(len(ws))
                if i == j or ws[i] <= 16 or ws[j] >= WMAX:
                    continue
                ws[i] -= 16
                ws[j] += 16
        elif op < 0.65 and ws:
            i = rng.randrange(len(ws))
            if ws[i] < 32:
                continue
            a = rng.randrange(1, ws[i] // 16) * 16
            ws = ws[:i] + [a, ws[i] - a] + ws[i + 1:]
        elif len(ws) >= 2:
            i = rng.randrange(len(ws))
            j = rng.randrange(len(ws))
            if i == j or ws[i] + ws[j] > WMAX:
                continue
            ws = [w for k2, w in enumerate(ws) if k2 not in (i, j)] + [ws[i] + ws[j]]
        else:
            continue
        if not all(16 <= w <= WMAX for w in ws) or sum(ws) < v[e]:
            continue
        ws.sort(reverse=True)
        old = cur[e]
        cur[e] = ws
        new_c = _sched_cost(cur)
        if new_c <= cur_c or rng.random() < np.exp((cur_c - new_c) / temp):
            cur_c = new_c
            if new_c < best_c:
                best_c = new_c
                best = [w[:] for w in cur]
        else:
            cur[e] = old
    return best


def _schedule(v_key):
    """v (tuple of 64 ints) -> (unit widths desc, assign[j][c] = (e, c0, cov)
    or None). All cores share the width sequence; piece rank r (pieces sorted
    desc) maps to unit r//8, core r%8, so piece width <= unit width."""
    if v_key in _SCHEDULE_CACHE:
        return _SCHEDULE_CACHE[v_key]
    v = list(v_key)
    cuts = _anneal(v)
    pieces = []  # (width, expert)
    for e, ws in enumerate(cuts):
        for w in ws:
            pieces.append((w, e))
    pieces.sort(key=lambda p: (-p[0], p[1]))
    widths = tuple(pieces[j][0] for j in range(0, len(pieces), 8))
    nunits = len(widths)
    # slots per expert: (unit_width, unit_idx, core)
    slots = {e: [] for e in range(len(v))}
    for r, (w, e) in enumerate(pieces):
        slots[e].append((widths[r // 8], r // 8, r % 8))
    assign = [[None] * N_CORES for _ in range(nunits)]
    for e, sl in slots.items():
        sl.sort(key=lambda s: -s[0])  # widest first
        cum = 0
        for wu, j, c in sl:
            if cum >= v[e]:
                continue  # overshoot slot -> dummy
            c0 = min(cum, CAP - wu)
            end = min(c0 + wu, v[e])
            assign[j][c] = (e, c0, end - c0)
            cum = end
        assert cum >= v[e], (e, v[e], sl)
    _SCHEDULE_CACHE[v_key] = (widths, assign)
    return widths, assign


# ----------------------------- device program -----------------------------

def _build_program(widths: tuple):
    nc = bacc.Bacc(None, target_bir_lowering=False)
    K = len(widths)

    xt = nc.dram_tensor("xt", [K, D, WMAX], BF16, kind="ExternalInput")
    w1g = nc.dram_tensor("w1g", [K, D, H], BF16, kind="ExternalInput")
    w2g = nc.dram_tensor("w2g", [K, H, D], BF16, kind="ExternalInput")
    b1g = nc.dram_tensor("b1g", [K, 128, MT1], F32, kind="ExternalInput")
    b2g = nc.dram_tensor("b2g", [K, 128, MT2], F32, kind="ExternalInput")
    yt = nc.dram_tensor("yt", [K, D, WMAX], F32, kind="ExternalOutput")
    scr = nc.dram_tensor("scr", [128, 64], F32, kind="ExternalOutput")

    Gelu = mybir.ActivationFunctionType.Gelu
    Ident = mybir.ActivationFunctionType.Identity

    # smallest unit first (its x+w arrive fastest -> compute starts early),
    # then descending so each unit's compute window covers the constant
    # ~4.2MB/unit weight-DMA debt; the first two emitted units stream w1 in
    # m-slices so GEMM1 proceeds per-slice while the rings ramp up
    emit_order = [K - 1] + list(range(K - 1))

    with tile.TileContext(nc) as tc:
        with (
            tc.tile_pool(name="wu", bufs=1) as wu,
            tc.tile_pool(name="w1p", bufs=2) as w1p,
            tc.tile_pool(name="w2p", bufs=2) as w2p,
            tc.tile_pool(name="bp", bufs=2) as bp,
            tc.tile_pool(name="xp", bufs=3) as xp,
            tc.tile_pool(name="hp", bufs=2) as hp,
            tc.tile_pool(name="yp", bufs=2) as yp,
            tc.tile_pool(name="ps_h", bufs=4, space="PSUM") as ps_h,
            tc.tile_pool(name="ps_y", bufs=4, space="PSUM") as ps_y,
        ):
            # PE pre-warm: ~5us of dummy matmuls during the initial DMA wait
            # flips the HAM clock gate to 8/8 before real work arrives
            warm = wu.tile([128, 64], BF16, tag="warm")
            nc.gpsimd.memset(warm, 0.0)
            wps = None
            for _ in range(96):
                wps = ps_h.tile([128, 64], F32, tag="psh")
                nc.tensor.matmul(
                    wps[:64, :], lhsT=warm, rhs=warm, start=True, stop=True)
            wout = wu.tile([128, 64], F32, tag="wout")
            nc.scalar.activation(wout[:64, :], wps[:64, :], Ident)
            nc.gpsimd.dma_start(out=scr[:64, :], in_=wout[:64, :])

            for ei, j in enumerate(emit_order):
                W = widths[j]
                first = ei < 2
                w1_src = w1g[j].rearrange("(k p) h -> p k h", p=128)
                w1_t = w1p.tile([128, KT1, H], BF16, tag="w1")
                x_t = xp.tile([128, KT1, WMAX], BF16, tag="x")
                if first:
                    # fine-grained early loads: compute starts per m-slice
                    # while the DMA rings are still ramping
                    nc.sync.dma_start(
                        out=w1_t[:, :, 0:128], in_=w1_src[:, :, 0:128])
                    nc.sync.dma_start(
                        out=x_t[:, :, :W],
                        in_=xt[j].rearrange("(k p) c -> p k c", p=128)[:, :, :W])
                    for m in range(1, MT1):
                        nc.sync.dma_start(
                            out=w1_t[:, :, m * 128:(m + 1) * 128],
                            in_=w1_src[:, :, m * 128:(m + 1) * 128])
                else:
                    nc.sync.dma_start(out=w1_t, in_=w1_src)
                    nc.sync.dma_start(
                        out=x_t[:, :, :W],
                        in_=xt[j].rearrange("(k p) c -> p k c", p=128)[:, :, :W])
                b1_t = bp.tile([128, MT1], F32, tag="b1")
                nc.scalar.dma_start(out=b1_t, in_=b1g[j])
                b2_t = bp.tile([128, MT2], F32, tag="b2")
                nc.scalar.dma_start(out=b2_t, in_=b2g[j])
                # w2 rides the second HWDGE ring (ACT): needed only for GEMM2
                w2_t = w2p.tile([128, KT2, D], BF16, tag="w2")
                nc.scalar.dma_start(
                    out=w2_t, in_=w2g[j].rearrange("(k p) d -> p k d", p=128))

                h_t = hp.tile([128, KT2, WMAX], BF16, tag="h")
                for m in range(MT1):
                    ps = ps_h.tile([128, WMAX], F32, tag="psh")
                    for k in range(KT1):
                        nc.tensor.matmul(
                            ps[:, :W],
                            lhsT=w1_t[:, k, m * 128:(m + 1) * 128],
                            rhs=x_t[:, k, :W],
                            start=(k == 0),
                            stop=(k == KT1 - 1),
                        )
                    nc.scalar.activation(
                        h_t[:, m, :W], ps[:, :W], Gelu, bias=b1_t[:, m:m + 1])

                y_t = yp.tile([128, MT2, WMAX], F32, tag="y")
                yt_s = yt[j].rearrange("(m p) c -> p m c", p=128)
                last = j == 0
                for dm in range(MT2):
                    ps2 = ps_y.tile([128, WMAX], F32, tag="psy")
                    for k in range(KT2):
                        nc.tensor.matmul(
                            ps2[:, :W],
                            lhsT=w2_t[:, k, dm * 128:(dm + 1) * 128],
                            rhs=h_t[:, k, :W],
                            start=(k == 0),
                            stop=(k == KT2 - 1),
                        )
                    nc.scalar.activation(
                        y_t[:, dm, :W], ps2[:, :W], Ident, bias=b2_t[:, dm:dm + 1])
                    if last:
                        # final unit: write out per m-tile to shrink the tail
                        nc.gpsimd.dma_start(
                            out=yt_s[:, dm, :W], in_=y_t[:, dm, :W])
                if not last:
                    nc.gpsimd.dma_start(
                        out=yt_s[:, :, :W], in_=y_t[:, :, :W])

    nc.compile()
    return nc


# ----------------------------- host wrapper -----------------------------

def kernel(packed_inputs, valid_load, w1, b1, w2, b2, _trace=False, **_):
    global LAST_RESULT
    packed_inputs = np.asarray(packed_inputs, np.float32)
    w1 = np.asarray(w1, np.float32)
    b1 = np.asarray(b1, np.float32)
    w2 = np.asarray(w2, np.float32)
    b2 = np.asarray(b2, np.float32)
    v = np.asarray(valid_load).astype(np.int64)

    out = np.zeros((E, CAP, D), np.float32)
    if int(v.max()) <= 0:
        return out

    widths, assign = _schedule(tuple(int(x) for x in v))
    K = len(widths)

    if widths not in _PROGRAM_CACHE:
        _PROGRAM_CACHE[widths] = _build_program(widths)
    nc = _PROGRAM_CACHE[widths]

    bf16 = mybir.dt.np(BF16)
    xt_all = np.ascontiguousarray(
        packed_inputs.transpose(0, 2, 1)).astype(bf16)      # [E, D, CAP]
    w1b = w1.astype(bf16)
    w2b = w2.astype(bf16)
    b1r = np.ascontiguousarray(
        b1.reshape(E, MT1, 128).transpose(0, 2, 1))          # [E, 128, MT1]
    b2r = np.ascontiguousarray(
        b2.reshape(E, MT2, 128).transpose(0, 2, 1))

    in_maps = []
    for c in range(N_CORES):
        xtc = np.zeros((K, D, WMAX), bf16)
        w1c = np.zeros((K, D, H), bf16)
        w2c = np.zeros((K, H, D), bf16)
        b1c = np.zeros((K, 128, MT1), np.float32)
        b2c = np.zeros((K, 128, MT2), np.float32)
        for j, W in enumerate(widths):
            pc = assign[j][c]
            if pc is None:
                continue
            e, c0, _cov = pc
            xtc[j, :, :W] = xt_all[e][:, c0:c0 + W]
            w1c[j] = w1b[e]
            w2c[j] = w2b[e]
            b1c[j] = b1r[e]
            b2c[j] = b2r[e]
        in_maps.append({"xt": xtc, "w1g": w1c, "w2g": w2c,
                        "b1g": b1c, "b2g": b2c})

    res = run_bass_kernel_spmd(nc, in_maps, list(range(N_CORES)), trace=_trace)
    LAST_RESULT = res

    for c in range(N_CORES):
        ytc = res.results[c]["yt"]
        for j in range(K):
            pc = assign[j][c]
            if pc is None:
                continue
            e, c0, cov = pc
            out[e, c0:c0 + cov, :] = ytc[j][:, :cov].T
    return out
